# revision 1
# baseline (speedup 1.0000x reference)
"""HONU order-3 kernel for 8 TRN2 NeuronCores.

Math: out[b] = sum_{i<=j<=k} w_ijk * xf_i * xf_j * xf_k,  xf = [1, x] (127 feats).

Restructuring: group combos by pair (i,j) (lex order => per-pair weights are a
contiguous slice of `weights`).  Let W[(i,j), k] = w_ijk for k>=j (0 otherwise).
Then  Z[b,(i,j)] = sum_k W[(i,j),k] * xf[b,k]   (a dense matmul), and
      out[b]     = sum_i xf_i * sum_{j>=i} xf_j * Z[b,(i,j)]
which maps onto one fused op per i-row (scalar_tensor_tensor):
      accum = sum_j ((Z * xf_i) * xf_j).

Sharding: pair-rows i are dealt round-robin to the 8 cores (core c gets rows
i = 8t + c, t = 0..15), so every core runs the same (SPMD) program: 16 fused
ops per 128-batch tile whose widths are padded to the 8-aligned grid
(row i covers j in [8*floor(i/8), 128); padding columns carry zero weights).
The fused ops are split between DVE and GPSIMD; ACT stages Z from PSUM to
SBUF (GPSIMD cannot read PSUM).  x is replicated; each core returns a [256,1]
partial that the host sums.

Matmuls run in float32r (full-rate fp32 PE mode); flip MM_F32R=False for
exact-fp32 (4x slower PE) if precision ever regresses.
"""

import numpy as np

import concourse.bass as bass
import concourse.bacc as bacc
import concourse.tile as tile
import concourse.mybir as mybir
from concourse.bass_utils import run_bass_kernel_spmd

F32 = mybir.dt.float32
F32R = mybir.dt.float32r
MM_F32R = True

P = 128
NF = 127            # features incl. bias
B = 256             # batch
NCLASS = 16         # width classes (i-rows per core)
WIDTHS = [128 - 8 * t for t in range(NCLASS)]           # 128,120,...,8
OFFS = np.concatenate([[0], np.cumsum(WIDTHS)])          # class col offsets
NCOLS = int(OFFS[-1])                                    # 1088
# chunk = (class range); each chunk is one matmul (N<=512)
CHUNKS = [(0, 4), (4, 9), (9, 16)]
CHUNK_COLS = [int(OFFS[hi] - OFFS[lo]) for lo, hi in CHUNKS]  # 464, 400, 224
GPS_CLASSES = set()   # GPSIMD cannot run TensorScalarPtr (walrus engine check)

_CACHE = {}


def _build_nc():
    mm_dt = F32R if MM_F32R else F32
    nc = bacc.Bacc("TRN2", target_bir_lowering=False, debug=False)
    xt = nc.dram_tensor("xt", [P, B], mm_dt, kind="ExternalInput")    # xf^T padded
    xb = nc.dram_tensor("xb", [B, P], F32, kind="ExternalInput")      # xf padded
    xs = nc.dram_tensor("xs", [B, NCLASS], F32, kind="ExternalInput")  # xf_i per class
    wds = [
        nc.dram_tensor(f"wd{ci}", [P, n], mm_dt, kind="ExternalInput")
        for ci, n in enumerate(CHUNK_COLS)
    ]
    out = nc.dram_tensor("out", [B, 1], F32, kind="ExternalOutput")

    with tile.TileContext(nc) as tc:
        with (
            tc.tile_pool(name="const", bufs=1) as cpool,
            tc.tile_pool(name="sb", bufs=2) as sb,
            tc.tile_pool(name="scrv", bufs=2) as scrv,
            tc.tile_pool(name="scrg", bufs=2) as scrg,
            tc.tile_pool(name="ps", bufs=2, space="PSUM") as ps,
        ):
            # spread loads over four HWDGE queues so the first matmul's
            # inputs (xt + wd0) land as early as possible
            xt_t = cpool.tile([P, B], mm_dt, tag="xt")
            nc.sync.dma_start(xt_t[:], xt[:])
            wd_t = [cpool.tile([P, n], mm_dt, tag=f"wd{ci}", name=f"wd{ci}_t")
                    for ci, n in enumerate(CHUNK_COLS)]
            nc.scalar.dma_start(wd_t[0][:], wds[0][:])
            nc.scalar.dma_start(wd_t[1][:], wds[1][:])
            nc.scalar.dma_start(wd_t[2][:], wds[2][:])
            xb_ts, xs_ts = [], []
            for bt in range(2):
                xb_t = sb.tile([P, P], F32, tag=f"xb{bt}", name=f"xb{bt}_t")
                nc.sync.dma_start(xb_t[:], xb[bt * P:(bt + 1) * P, :])
                xs_t = sb.tile([P, NCLASS], F32, tag=f"xs{bt}", name=f"xs{bt}_t")
                nc.sync.dma_start(xs_t[:], xs[bt * P:(bt + 1) * P, :])
                xb_ts.append(xb_t)
                xs_ts.append(xs_t)

            for bt in range(2):
                xb_t, xs_t = xb_ts[bt], xs_ts[bt]
                g = sb.tile([P, NCLASS], F32, tag=f"g{bt}", name=f"g{bt}_t")
                for ci, (lo, hi) in enumerate(CHUNKS):
                    n = CHUNK_COLS[ci]
                    z_ps = ps.tile([P, n], F32, tag=f"z{ci}", name=f"z{ci}_ps")
                    nc.tensor.matmul(
                        z_ps[:], xt_t[:, bt * P:(bt + 1) * P], wd_t[ci][:],
                        start=True, stop=True,
                    )
                    z_sb = sb.tile([P, n], F32, tag=f"zsb{ci}", name=f"z{ci}_sb")
                    nc.scalar.copy(z_sb[:], z_ps[:])
                    for t in range(lo, hi):
                        w = WIDTHS[t]
                        o = int(OFFS[t] - OFFS[lo])
                        eng = nc.gpsimd if t in GPS_CLASSES else nc.vector
                        pool = scrg if t in GPS_CLASSES else scrv
                        s = pool.tile([P, 128], F32, tag="s", name="s_t")
                        eng.scalar_tensor_tensor(
                            out=s[:, :w],
                            in0=z_sb[:, o:o + w],
                            scalar=xs_t[:, t:t + 1],
                            in1=xb_t[:, 8 * t:8 * t + w],
                            op0=mybir.AluOpType.mult,
                            op1=mybir.AluOpType.mult,
                            accum_out=g[:, t:t + 1],
                        )
                res = sb.tile([P, 1], F32, tag=f"res{bt}", name=f"res{bt}_t")
                nc.vector.reduce_sum(res[:], g[:], axis=mybir.AxisListType.X)
                nc.sync.dma_start(out[bt * P:(bt + 1) * P, :], res[:])
    nc.compile()
    return nc


def _prep_inputs(x, weights, comb_idx):
    """Host-side layout prep (no FLOPs on the runtime data beyond zero-fill
    scatter): build xf paddings and the per-core dense weight chunks."""
    x = np.ascontiguousarray(np.asarray(x, dtype=np.float32))
    w = np.asarray(weights, dtype=np.float32).ravel()
    ci = np.asarray(comb_idx)
    i_, j_ = ci[:, 0].astype(np.int64), ci[:, 1].astype(np.int64)
    k_ = ci[:, 2].astype(np.int64)

    xf = np.concatenate([np.ones((B, 1), np.float32), x], axis=1)   # [256,127]
    xb = np.zeros((B, P), np.float32)
    xb[:, :NF] = xf
    xt = np.zeros((P, B), np.float32)
    xt[:NF, :] = xf.T

    # lex pair-row index of each combo
    ar = np.arange(NF, dtype=np.int64)
    rsp = ar * NF - (ar * (ar - 1)) // 2
    q = rsp[i_] + (j_ - i_)
    Wd = np.zeros((8128, NF), np.float32)
    Wd[q, k_] = w

    in_maps = []
    for c in range(8):
        big = np.zeros((P, NCOLS), np.float32)
        xs = np.zeros((B, NCLASS), np.float32)
        for t in range(NCLASS):
            i = 8 * t + c
            if i > 126:
                continue
            xs[:, t] = xf[:, i]
            p0 = int(rsp[i])
            # cols j in [i,127) hold Wd rows p0..p0+(127-i); leading j in
            # [8t, i) and trailing j=127 stay zero
            o = int(OFFS[t])
            big[:NF, o + (i - 8 * t): o + (127 - 8 * t)] = Wd[p0:p0 + (NF - i)].T
        m = {"xt": xt, "xb": xb, "xs": xs}
        for ci2, (lo, hi) in enumerate(CHUNKS):
            m[f"wd{ci2}"] = np.ascontiguousarray(
                big[:, int(OFFS[lo]):int(OFFS[hi])])
        in_maps.append(m)
    return in_maps


def _get_nc():
    if "nc" not in _CACHE:
        _CACHE["nc"] = _build_nc()
    return _CACHE["nc"]


def run_spmd(x, weights, comb_idx, trace=False):
    nc = _get_nc()
    in_maps = _prep_inputs(x, weights, comb_idx)
    res = run_bass_kernel_spmd(nc, in_maps, list(range(8)), trace=trace)
    acc = np.zeros((B, 1), np.float64)
    for c in range(8):
        acc += res.results[c]["out"].astype(np.float64)
    return acc.astype(np.float32), res


def kernel(x, weights, comb_idx):
    out, _ = run_spmd(x, weights, comb_idx, trace=False)
    return out



# revision 3
# speedup vs baseline: 1.0768x; 1.0768x over previous
"""HONU order-3 kernel for 8 TRN2 NeuronCores.

Math: out[b] = sum_{i<=j<=k} w_ijk * xf_i * xf_j * xf_k,  xf = [1, x] (127 feats).

Restructuring: group combos by pair (i,j) (lex order => per-pair weights are a
contiguous slice of `weights`).  Let W[(i,j), k] = w_ijk for k>=j (0 otherwise).
Then  Z[b,(i,j)] = sum_k W[(i,j),k] * xf[b,k]   (a dense matmul), and
      out[b]     = sum_i xf_i * sum_{j>=i} xf_j * Z[b,(i,j)]
mapped onto one fused scalar_tensor_tensor per i-row:
      g[:, t] = sum_j ((Z * xf_i) * xf_j).

Sharding: pair-rows i are dealt round-robin to the 8 cores (core c gets rows
i = 8t + c, t = 0..15), so every core runs the same (SPMD) program.  Each core
returns a [128, 2] partial (batch tile x 1) that the host sums across cores.

v2 changes vs the 28.5us baseline:
  * inputs consolidated into 4 DMAs with large per-partition rows (the old
    7-DMA layout moved 837KB as ~1440 sub-1KB packets, ~14us of DMA time)
  * bf16 for matmul operands and elementwise operands (halves DMA bytes and
    doubles DVE throughput; fp32 accumulation everywhere)
  * per-class fused ops split between DVE and GPSIMD
  * single [128, 2] output DMA instead of two [128, 1] DMAs
"""

import numpy as np
import ml_dtypes

import concourse.bass as bass
import concourse.bacc as bacc
import concourse.tile as tile
import concourse.mybir as mybir
from concourse.bass_utils import run_bass_kernel_spmd

F32 = mybir.dt.float32
BF16 = mybir.dt.bfloat16
NPBF16 = np.dtype(ml_dtypes.bfloat16)

P = 128
NF = 127            # features incl. bias
B = 256             # batch
NCLASS = 16         # width classes (i-rows per core)
WIDTHS = [128 - 8 * t for t in range(NCLASS)]           # 128,120,...,8
OFFS = np.concatenate([[0], np.cumsum(WIDTHS)])          # class col offsets
NCOLS = int(OFFS[-1])                                    # 1088
# chunk = (class range); each chunk is one matmul (N<=512, one PSUM bank)
CHUNKS = [(0, 4), (4, 9), (9, 16)]
CHUNK_COLS = [int(OFFS[hi] - OFFS[lo]) for ci, (lo, hi) in enumerate(CHUNKS)]
# classes handled on GPSIMD (rest on DVE).  Empty: walrus rejects
# TensorScalarPtr on Pool (engine check at NEFF codegen).
GPS_CLASSES = set()

_CACHE = {}


def _build_nc():
    nc = bacc.Bacc("TRN2", target_bir_lowering=False, debug=False)
    xt = nc.dram_tensor("xt", [P, B], BF16, kind="ExternalInput")      # xf^T padded
    wd = nc.dram_tensor("wd", [P, NCOLS], BF16, kind="ExternalInput")  # dense pair weights
    xb = nc.dram_tensor("xb", [P, B], BF16, kind="ExternalInput")      # xf rows, 2 batch tiles
    xs = nc.dram_tensor("xs", [P, 2 * NCLASS], F32, kind="ExternalInput")  # xf_i per class
    out = nc.dram_tensor("out", [P, 2], F32, kind="ExternalOutput")

    with tile.TileContext(nc) as tc:
        with (
            tc.tile_pool(name="const", bufs=1) as cpool,
            tc.tile_pool(name="sb", bufs=2) as sb,
            tc.tile_pool(name="scrv", bufs=2) as scrv,
            tc.tile_pool(name="scrg", bufs=2) as scrg,
            tc.tile_pool(name="ps", bufs=2, space="PSUM") as ps,
        ):
            xt_t = cpool.tile([P, B], BF16, tag="xt")
            nc.sync.dma_start(xt_t[:], xt[:])
            wd_t = cpool.tile([P, NCOLS], BF16, tag="wd")
            nc.sync.dma_start(wd_t[:], wd[:])
            xb_t = cpool.tile([P, B], BF16, tag="xb")
            nc.scalar.dma_start(xb_t[:], xb[:])
            xs_t = cpool.tile([P, 2 * NCLASS], F32, tag="xs")
            nc.scalar.dma_start(xs_t[:], xs[:])

            g = cpool.tile([P, 2 * NCLASS], F32, tag="g")
            res = cpool.tile([P, 2], F32, tag="res")

            for bt in range(2):
                z_sb = sb.tile([P, NCOLS], BF16, tag=f"z{bt}", name=f"z{bt}_sb")
                for ci, (lo, hi) in enumerate(CHUNKS):
                    n = CHUNK_COLS[ci]
                    o = int(OFFS[lo])
                    z_ps = ps.tile([P, n], F32, tag=f"z{ci}", name=f"z{ci}_ps")
                    nc.tensor.matmul(
                        z_ps[:], xt_t[:, bt * P:(bt + 1) * P], wd_t[:, o:o + n],
                        start=True, stop=True,
                    )
                    nc.scalar.copy(z_sb[:, o:o + n], z_ps[:])
                for t in range(NCLASS):
                    w = WIDTHS[t]
                    o = int(OFFS[t])
                    eng = nc.gpsimd if t in GPS_CLASSES else nc.vector
                    pool = scrg if t in GPS_CLASSES else scrv
                    s = pool.tile([P, P], BF16, tag="s", name="s_t")
                    eng.scalar_tensor_tensor(
                        out=s[:, :w],
                        in0=z_sb[:, o:o + w],
                        scalar=xs_t[:, bt * NCLASS + t:bt * NCLASS + t + 1],
                        in1=xb_t[:, bt * P + 8 * t:bt * P + 8 * t + w],
                        op0=mybir.AluOpType.mult,
                        op1=mybir.AluOpType.mult,
                        accum_out=g[:, bt * NCLASS + t:bt * NCLASS + t + 1],
                    )
                nc.vector.reduce_sum(
                    res[:, bt:bt + 1], g[:, bt * NCLASS:(bt + 1) * NCLASS],
                    axis=mybir.AxisListType.X,
                )
            nc.sync.dma_start(out[:], res[:])
    nc.compile()
    return nc


def _prep_inputs(x, weights, comb_idx):
    """Host-side layout prep (no FLOPs on the runtime data beyond zero-fill
    scatter): build xf paddings and the per-core dense weight chunks."""
    x = np.ascontiguousarray(np.asarray(x, dtype=np.float32))
    w = np.asarray(weights, dtype=np.float32).ravel()
    ci = np.asarray(comb_idx)
    i_, j_ = ci[:, 0].astype(np.int64), ci[:, 1].astype(np.int64)
    k_ = ci[:, 2].astype(np.int64)

    xf = np.concatenate([np.ones((B, 1), np.float32), x], axis=1)   # [256,127]
    # xb: [128, 256] bf16 - row p holds xf[p, :] | xf[128+p, :]
    xb = np.zeros((P, B), np.float32)
    xb[:, :NF] = xf[:P, :]
    xb[:, P:P + NF] = xf[P:, :]
    xt = np.zeros((P, B), np.float32)
    xt[:NF, :] = xf.T

    # lex pair-row index of each combo
    ar = np.arange(NF, dtype=np.int64)
    rsp = ar * NF - (ar * (ar - 1)) // 2
    q = rsp[i_] + (j_ - i_)
    Wd = np.zeros((8128, NF), np.float32)
    Wd[q, k_] = w

    xt_bf = xt.astype(NPBF16)
    xb_bf = xb.astype(NPBF16)

    in_maps = []
    for c in range(8):
        big = np.zeros((P, NCOLS), np.float32)
        xs = np.zeros((P, 2 * NCLASS), np.float32)
        for t in range(NCLASS):
            i = 8 * t + c
            if i > 126:
                continue
            xs[:, t] = xf[:P, i]
            xs[:, NCLASS + t] = xf[P:, i]
            p0 = int(rsp[i])
            # cols j in [i,127) hold Wd rows p0..p0+(127-i); leading j in
            # [8t, i) and trailing j=127 stay zero
            o = int(OFFS[t])
            big[:NF, o + (i - 8 * t): o + (127 - 8 * t)] = Wd[p0:p0 + (NF - i)].T
        m = {
            "xt": xt_bf, "xb": xb_bf, "xs": xs,
            "wd": big.astype(NPBF16),
        }
        in_maps.append(m)
    return in_maps


def _get_nc():
    if "nc" not in _CACHE:
        _CACHE["nc"] = _build_nc()
    return _CACHE["nc"]


def run_spmd(x, weights, comb_idx, trace=False):
    nc = _get_nc()
    in_maps = _prep_inputs(x, weights, comb_idx)
    res = run_bass_kernel_spmd(nc, in_maps, list(range(8)), trace=trace)
    acc = np.zeros((B, 1), np.float64)
    for c in range(8):
        r = res.results[c]["out"].astype(np.float64)   # [128, 2]
        acc[:P, 0] += r[:, 0]
        acc[P:, 0] += r[:, 1]
    return acc.astype(np.float32), res


def kernel(x, weights, comb_idx):
    out, _ = run_spmd(x, weights, comb_idx, trace=False)
    return out


# revision 5
# speedup vs baseline: 1.2509x; 1.1617x over previous
"""HONU order-3 kernel for 8 TRN2 NeuronCores — raw bass (no TileContext).

Math: out[b] = sum_{i<=j<=k} w_ijk * xf_i * xf_j * xf_k,  xf = [1, x] (127 feats).

Restructuring: group combos by pair (i,j) (lex order => per-pair weights are a
contiguous slice of `weights`).  Let W[(i,j), k] = w_ijk for k>=j (0 otherwise).
Then  Z[b,(i,j)] = sum_k W[(i,j),k] * xf[b,k]   (a dense matmul), and
      out[b]     = sum_i xf_i * sum_{j>=i} xf_j * Z[b,(i,j)]
mapped onto one fused scalar_tensor_tensor per i-row and batch tile:
      g[:, t] = sum_j ((Z * xf_i) * xf_j).

Sharding: pair-rows i are dealt round-robin to the 8 cores (core c gets rows
i = 8t + c, t = 0..15); every core runs the same (SPMD) program and returns a
[128, 2] partial (batch-tile column-major) that the host sums across cores.

Why raw bass: the TileContext version spent ~10us in framework preamble +
semaphore-teardown epilogue (50+ sems cleared one-by-one) and ~4us issuing 7
small DMAs.  Here: 5 consolidated DMAs (>=512B per-partition rows), 7 manual
semaphores cleared with one range-clear, bf16 matmul operands (halves weight
DMA; full-rate PE), fp32 elementwise (STT has no 2x bf16 mode - measured).
"""

import numpy as np
import ml_dtypes

import concourse.bass as bass
import concourse.bacc as bacc
import concourse.mybir as mybir
from concourse.bass_utils import run_bass_kernel_spmd

F32 = mybir.dt.float32
BF16 = mybir.dt.bfloat16
NPBF16 = np.dtype(ml_dtypes.bfloat16)

P = 128
NF = 127            # features incl. bias
B = 256             # batch
NCLASS = 16         # width classes (i-rows per core)
WIDTHS = [128 - 8 * t for t in range(NCLASS)]           # 128,120,...,8
OFFS = np.concatenate([[0], np.cumsum(WIDTHS)])          # class col offsets
NCOLS = int(OFFS[-1])                                    # 1088
# chunk = (class range); each chunk is one matmul (N<=512, one PSUM bank)
CHUNKS = [(0, 4), (4, 9), (9, 16)]
CHUNK_COLS = [int(OFFS[hi] - OFFS[lo]) for lo, hi in CHUNKS]      # 464,400,224
# class -> chunk index
CLASS_CHUNK = {t: ci for ci, (lo, hi) in enumerate(CHUNKS) for t in range(lo, hi)}

_CACHE = {}


def _build_nc():
    nc = bacc.Bacc("TRN2", target_bir_lowering=False, debug=False)
    xt = nc.dram_tensor("xt", [P, B], BF16, kind="ExternalInput")      # xf^T padded
    wd = nc.dram_tensor("wd", [P, NCOLS], BF16, kind="ExternalInput")  # dense pair weights
    xb = nc.dram_tensor("xb", [P, B], F32, kind="ExternalInput")       # xf rows, 2 tiles
    xs = nc.dram_tensor("xs", [P, 2 * NCLASS], F32, kind="ExternalInput")
    out = nc.dram_tensor("out", [P, 2], F32, kind="ExternalOutput")

    from contextlib import ExitStack
    with ExitStack() as ctx:
        ctx.enter_context(nc.cleanup_on_exit())
        xt_t = ctx.enter_context(nc.sbuf_tensor("xt_t", [P, B], BF16))
        wd_t = ctx.enter_context(nc.sbuf_tensor("wd_t", [P, NCOLS], BF16))
        xb_t = ctx.enter_context(nc.sbuf_tensor("xb_t", [P, B], F32))
        xs_t = ctx.enter_context(nc.sbuf_tensor("xs_t", [P, 2 * NCLASS], F32))
        z0_sb = ctx.enter_context(nc.sbuf_tensor("z0_sb", [P, NCOLS], F32))
        z1_sb = ctx.enter_context(nc.sbuf_tensor("z1_sb", [P, NCOLS], F32))
        s_t = ctx.enter_context(nc.sbuf_tensor("s_t", [P, P], F32))
        g_t = ctx.enter_context(nc.sbuf_tensor("g_t", [P, 2 * NCLASS], F32))
        res_t = ctx.enter_context(nc.sbuf_tensor("res_t", [P, 2], F32))
        z00 = ctx.enter_context(nc.psum_tensor("z00", [P, CHUNK_COLS[0]], F32))
        z01 = ctx.enter_context(nc.psum_tensor("z01", [P, CHUNK_COLS[1]], F32))
        z02 = ctx.enter_context(nc.psum_tensor("z02", [P, CHUNK_COLS[2]], F32))
        z10 = ctx.enter_context(nc.psum_tensor("z10", [P, CHUNK_COLS[0]], F32))
        z11 = ctx.enter_context(nc.psum_tensor("z11", [P, CHUNK_COLS[1]], F32))
        z12 = ctx.enter_context(nc.psum_tensor("z12", [P, CHUNK_COLS[2]], F32))
        s_xt = ctx.enter_context(nc.semaphore("s_xt"))
        s_wd = ctx.enter_context(nc.semaphore("s_wd"))
        s_xbs = ctx.enter_context(nc.semaphore("s_xbs"))
        s_mm = ctx.enter_context(nc.semaphore("s_mm"))
        s_act = ctx.enter_context(nc.semaphore("s_act"))
        s_dve = ctx.enter_context(nc.semaphore("s_dve"))
        s_out = ctx.enter_context(nc.semaphore("s_out"))
        block = ctx.enter_context(nc.Block())
        if True:
            z_sb = [z0_sb, z1_sb]
            z_ps = [[z00, z01, z02], [z10, z11, z12]]

            @block.sync
            def _(sync):
                sync.dma_start(xt_t[:], xt[:]).then_inc(s_xt, 16)
                c0 = CHUNK_COLS[0]
                sync.dma_start(wd_t[:, 0:c0], wd[:, 0:c0]).then_inc(s_wd, 16)
                sync.dma_start(wd_t[:, c0:NCOLS], wd[:, c0:NCOLS]).then_inc(
                    s_wd, 16
                )
                sync.wait_ge(s_dve, 1)
                sync.dma_start(out[:], res_t[:]).then_inc(s_out, 16)
                sync.wait_ge(s_out, 16)

            @block.scalar
            def _(scalar):
                scalar.dma_start(xb_t[:], xb[:]).then_inc(s_xbs, 16)
                scalar.dma_start(xs_t[:], xs[:]).then_inc(s_xbs, 16)
                for bt in range(2):
                    for ci in range(3):
                        n = CHUNK_COLS[ci]
                        o = int(OFFS[CHUNKS[ci][0]])
                        scalar.wait_ge(s_mm, bt * 3 + ci + 1)
                        scalar.copy(
                            z_sb[bt][:, o:o + n], z_ps[bt][ci][:]
                        ).then_inc(s_act, 1)

            @block.tensor
            def _(tensor):
                tensor.wait_ge(s_xt, 16)
                for bt in range(2):
                    for ci in range(3):
                        n = CHUNK_COLS[ci]
                        o = int(OFFS[CHUNKS[ci][0]])
                        if bt == 0 and ci == 0:
                            tensor.wait_ge(s_wd, 16)
                        elif bt == 0 and ci == 1:
                            tensor.wait_ge(s_wd, 32)
                        tensor.matmul(
                            z_ps[bt][ci][:],
                            xt_t[:, bt * P:(bt + 1) * P],
                            wd_t[:, o:o + n],
                            start=True, stop=True,
                        ).then_inc(s_mm, 1)

            @block.vector
            def _(vector):
                vector.wait_ge(s_xbs, 32)
                for bt in range(2):
                    for t in range(NCLASS):
                        w = WIDTHS[t]
                        o = int(OFFS[t])
                        if t in (0, 4, 9):
                            vector.wait_ge(s_act, bt * 3 + CLASS_CHUNK[t] + 1)
                        vector.scalar_tensor_tensor(
                            out=s_t[:, :w],
                            in0=z_sb[bt][:, o:o + w],
                            scalar=xs_t[:, bt * NCLASS + t:bt * NCLASS + t + 1],
                            in1=xb_t[:, bt * P + 8 * t:bt * P + 8 * t + w],
                            op0=mybir.AluOpType.mult,
                            op1=mybir.AluOpType.mult,
                            accum_out=g_t[:, bt * NCLASS + t:bt * NCLASS + t + 1],
                        )
                vector.reduce_sum(
                    res_t[:],
                    g_t[:].rearrange("p (b t) -> p b t", b=2),
                    axis=mybir.AxisListType.X,
                ).then_inc(s_dve, 1)

    nc.compile()
    return nc


def _prep_inputs(x, weights, comb_idx):
    """Host-side layout prep (no FLOPs on the runtime data beyond zero-fill
    scatter): build xf paddings and the per-core dense weight chunks."""
    x = np.ascontiguousarray(np.asarray(x, dtype=np.float32))
    w = np.asarray(weights, dtype=np.float32).ravel()
    ci = np.asarray(comb_idx)
    i_, j_ = ci[:, 0].astype(np.int64), ci[:, 1].astype(np.int64)
    k_ = ci[:, 2].astype(np.int64)

    xf = np.concatenate([np.ones((B, 1), np.float32), x], axis=1)   # [256,127]
    xbm = np.zeros((P, B), np.float32)       # row p: xf[p,:] | xf[128+p,:]
    xbm[:, :NF] = xf[:P, :]
    xbm[:, P:P + NF] = xf[P:, :]
    xt = np.zeros((P, B), np.float32)
    xt[:NF, :] = xf.T

    # lex pair-row index of each combo
    ar = np.arange(NF, dtype=np.int64)
    rsp = ar * NF - (ar * (ar - 1)) // 2
    q = rsp[i_] + (j_ - i_)
    Wd = np.zeros((8128, NF), np.float32)
    Wd[q, k_] = w

    xt_bf = xt.astype(NPBF16)

    in_maps = []
    for c in range(8):
        big = np.zeros((P, NCOLS), np.float32)
        xsm = np.zeros((P, 2 * NCLASS), np.float32)
        for t in range(NCLASS):
            i = 8 * t + c
            if i > 126:
                continue
            xsm[:, t] = xf[:P, i]
            xsm[:, NCLASS + t] = xf[P:, i]
            p0 = int(rsp[i])
            # cols j in [i,127) hold Wd rows p0..p0+(127-i); leading j in
            # [8t, i) and trailing j=127 stay zero
            o = int(OFFS[t])
            big[:NF, o + (i - 8 * t): o + (127 - 8 * t)] = Wd[p0:p0 + (NF - i)].T
        m = {"xt": xt_bf, "xb": xbm, "xs": xsm, "wd": big.astype(NPBF16)}
        in_maps.append(m)
    return in_maps


def _get_nc():
    if "nc" not in _CACHE:
        _CACHE["nc"] = _build_nc()
    return _CACHE["nc"]


def run_spmd(x, weights, comb_idx, trace=False):
    nc = _get_nc()
    in_maps = _prep_inputs(x, weights, comb_idx)
    res = run_bass_kernel_spmd(nc, in_maps, list(range(8)), trace=trace)
    acc = np.zeros((B, 1), np.float64)
    for c in range(8):
        r = res.results[c]["out"].astype(np.float64)   # [128, 2]
        acc[:P, 0] += r[:, 0]
        acc[P:, 0] += r[:, 1]
    return acc.astype(np.float32), res


def kernel(x, weights, comb_idx):
    out, _ = run_spmd(x, weights, comb_idx, trace=False)
    return out


# revision 7
# speedup vs baseline: 1.3424x; 1.0731x over previous
"""HONU order-3 kernel for 8 TRN2 NeuronCores — raw bass (no TileContext).

Math: out[b] = sum_{i<=j<=k} w_ijk * xf_i * xf_j * xf_k,  xf = [1, x] (127 feats).

Restructuring: group combos by pair (i,j) (lex order => per-pair weights are a
contiguous slice of `weights`).  Let W[(i,j), k] = w_ijk for k>=j (0 otherwise).
Then  Z[b,(i,j)] = sum_k W[(i,j),k] * xf[b,k]   (a dense matmul), and
      out[b]     = sum_i xf_i * sum_{j>=i} xf_j * Z[b,(i,j)]
mapped onto one fused scalar_tensor_tensor per i-row and batch tile:
      g[:, t] = sum_j ((Z * xf_i) * xf_j).

Sharding: pair-rows i are dealt round-robin to the 8 cores (core c gets rows
i = 8t + c, t = 0..15); every core runs the same (SPMD) program and returns a
[128, 2] partial (batch-tile column-major) that the host sums across cores.

Why raw bass: the TileContext version spent ~10us in framework preamble +
semaphore-teardown epilogue (50+ sems cleared one-by-one) and ~4us issuing 7
small DMAs.  Here: 5 consolidated DMAs (>=512B per-partition rows), 7 manual
semaphores cleared with one range-clear, bf16 matmul operands (halves weight
DMA; full-rate PE), fp32 elementwise (STT has no 2x bf16 mode - measured).
"""

import numpy as np
import ml_dtypes

import concourse.bass as bass
import concourse.bacc as bacc
import concourse.mybir as mybir
from concourse.bass_utils import run_bass_kernel_spmd

F32 = mybir.dt.float32
BF16 = mybir.dt.bfloat16
NPBF16 = np.dtype(ml_dtypes.bfloat16)

P = 128
NF = 127            # features incl. bias
B = 256             # batch
NCLASS = 16         # width classes (i-rows per core)
WIDTHS = [128 - 8 * t for t in range(NCLASS)]           # 128,120,...,8
OFFS = np.concatenate([[0], np.cumsum(WIDTHS)])          # class col offsets
NCOLS = int(OFFS[-1])                                    # 1088
# chunk = (class range); each chunk is one matmul (N<=512, one PSUM bank)
CHUNKS = [(0, 4), (4, 9), (9, 16)]
CHUNK_COLS = [int(OFFS[hi] - OFFS[lo]) for lo, hi in CHUNKS]      # 464,400,224
# class -> chunk index
CLASS_CHUNK = {t: ci for ci, (lo, hi) in enumerate(CHUNKS) for t in range(lo, hi)}

_CACHE = {}


def _build_nc():
    nc = bacc.Bacc("TRN2", target_bir_lowering=False, debug=False)
    xt = nc.dram_tensor("xt", [P, B], BF16, kind="ExternalInput")      # xf^T padded
    wd = nc.dram_tensor("wd", [P, NCOLS], BF16, kind="ExternalInput")  # dense pair weights
    xb = nc.dram_tensor("xb", [P, B], F32, kind="ExternalInput")       # xf rows, 2 tiles
    xs = nc.dram_tensor("xs", [P, 2 * NCLASS], F32, kind="ExternalInput")
    out = nc.dram_tensor("out", [P, 2], F32, kind="ExternalOutput")

    from contextlib import ExitStack
    with ExitStack() as ctx:
        xt_t = ctx.enter_context(nc.sbuf_tensor("xt_t", [P, B], BF16))
        wd_t = ctx.enter_context(nc.sbuf_tensor("wd_t", [P, NCOLS], BF16))
        xb_t = ctx.enter_context(nc.sbuf_tensor("xb_t", [P, B], F32))
        xs_t = ctx.enter_context(nc.sbuf_tensor("xs_t", [P, 2 * NCLASS], F32))
        z0_sb = ctx.enter_context(nc.sbuf_tensor("z0_sb", [P, NCOLS], F32))
        z1_sb = ctx.enter_context(nc.sbuf_tensor("z1_sb", [P, NCOLS], F32))
        s_t = ctx.enter_context(nc.sbuf_tensor("s_t", [P, P], F32))
        g_t = ctx.enter_context(nc.sbuf_tensor("g_t", [P, 2 * NCLASS], F32))
        res_t = ctx.enter_context(nc.sbuf_tensor("res_t", [P, 2], F32))
        z00 = ctx.enter_context(nc.psum_tensor("z00", [P, CHUNK_COLS[0]], F32))
        z01 = ctx.enter_context(nc.psum_tensor("z01", [P, CHUNK_COLS[1]], F32))
        z02 = ctx.enter_context(nc.psum_tensor("z02", [P, CHUNK_COLS[2]], F32))
        z10 = ctx.enter_context(nc.psum_tensor("z10", [P, CHUNK_COLS[0]], F32))
        z11 = ctx.enter_context(nc.psum_tensor("z11", [P, CHUNK_COLS[1]], F32))
        z12 = ctx.enter_context(nc.psum_tensor("z12", [P, CHUNK_COLS[2]], F32))
        s_xt = ctx.enter_context(nc.semaphore("s_xt"))
        s_wd = ctx.enter_context(nc.semaphore("s_wd"))
        s_xbs = ctx.enter_context(nc.semaphore("s_xbs"))
        s_mm = ctx.enter_context(nc.semaphore("s_mm"))
        s_act = ctx.enter_context(nc.semaphore("s_act"))
        s_dve = ctx.enter_context(nc.semaphore("s_dve"))
        s_out = ctx.enter_context(nc.semaphore("s_out"))

        z_sb = [z0_sb, z1_sb]
        z_ps = [[z00, z01, z02], [z10, z11, z12]]

        # No nc.Block(): engines end independently (no final all-engine
        # barrier), so the walrus per-engine semaphore-file reset epilogue
        # (~50 sem writes per engine) overlaps the DVE phase on the engines
        # that finish early instead of trailing the whole kernel.  It also
        # re-zeroes our 7 sems for the next execution.
        c0 = CHUNK_COLS[0]

        # DMA issues (sync ring: weights; scalar ring: xt + xb + xs)
        nc.sync.dma_start(wd_t[:, 0:c0], wd[:, 0:c0]).then_inc(s_wd, 16)
        nc.scalar.dma_start(xt_t[:], xt[:]).then_inc(s_xt, 16)
        nc.sync.dma_start(wd_t[:, c0:NCOLS], wd[:, c0:NCOLS]).then_inc(s_wd, 16)
        nc.scalar.dma_start(xb_t[:], xb[:]).then_inc(s_xbs, 16)
        nc.scalar.dma_start(xs_t[:], xs[:]).then_inc(s_xbs, 16)

        # PE: 6 matmuls
        nc.tensor.wait_ge(s_xt, 16)
        for bt in range(2):
            for ci in range(3):
                n = CHUNK_COLS[ci]
                o = int(OFFS[CHUNKS[ci][0]])
                if bt == 0 and ci == 0:
                    nc.tensor.wait_ge(s_wd, 16)
                elif bt == 0 and ci == 1:
                    nc.tensor.wait_ge(s_wd, 32)
                nc.tensor.matmul(
                    z_ps[bt][ci][:],
                    xt_t[:, bt * P:(bt + 1) * P],
                    wd_t[:, o:o + n],
                    start=True, stop=True,
                ).then_inc(s_mm, 1)

        # ACT: 6 PSUM->SBUF chunk copies
        for bt in range(2):
            for ci in range(3):
                n = CHUNK_COLS[ci]
                o = int(OFFS[CHUNKS[ci][0]])
                nc.scalar.wait_ge(s_mm, bt * 3 + ci + 1)
                nc.scalar.copy(z_sb[bt][:, o:o + n], z_ps[bt][ci][:]).then_inc(
                    s_act, 1
                )

        # DVE: 32 fused per-class ops + one reduce
        nc.vector.wait_ge(s_xbs, 32)
        for bt in range(2):
            for t in range(NCLASS):
                w = WIDTHS[t]
                o = int(OFFS[t])
                if t in (0, 4, 9):
                    nc.vector.wait_ge(s_act, bt * 3 + CLASS_CHUNK[t] + 1)
                nc.vector.scalar_tensor_tensor(
                    out=s_t[:, :w],
                    in0=z_sb[bt][:, o:o + w],
                    scalar=xs_t[:, bt * NCLASS + t:bt * NCLASS + t + 1],
                    in1=xb_t[:, bt * P + 8 * t:bt * P + 8 * t + w],
                    op0=mybir.AluOpType.mult,
                    op1=mybir.AluOpType.mult,
                    accum_out=g_t[:, bt * NCLASS + t:bt * NCLASS + t + 1],
                )
        nc.vector.reduce_sum(
            res_t[:],
            g_t[:].rearrange("p (b t) -> p b t", b=2),
            axis=mybir.AxisListType.X,
        ).then_inc(s_dve, 1)

        # output DMA + completion gate
        nc.sync.wait_ge(s_dve, 1)
        nc.sync.dma_start(out[:], res_t[:]).then_inc(s_out, 16)
        nc.sync.wait_ge(s_out, 16)

    nc.compile()
    return nc


def _prep_inputs(x, weights, comb_idx):
    """Host-side layout prep (no FLOPs on the runtime data beyond zero-fill
    scatter): build xf paddings and the per-core dense weight chunks."""
    x = np.ascontiguousarray(np.asarray(x, dtype=np.float32))
    w = np.asarray(weights, dtype=np.float32).ravel()
    ci = np.asarray(comb_idx)
    i_, j_ = ci[:, 0].astype(np.int64), ci[:, 1].astype(np.int64)
    k_ = ci[:, 2].astype(np.int64)

    xf = np.concatenate([np.ones((B, 1), np.float32), x], axis=1)   # [256,127]
    xbm = np.zeros((P, B), np.float32)       # row p: xf[p,:] | xf[128+p,:]
    xbm[:, :NF] = xf[:P, :]
    xbm[:, P:P + NF] = xf[P:, :]
    xt = np.zeros((P, B), np.float32)
    xt[:NF, :] = xf.T

    # lex pair-row index of each combo
    ar = np.arange(NF, dtype=np.int64)
    rsp = ar * NF - (ar * (ar - 1)) // 2
    q = rsp[i_] + (j_ - i_)
    Wd = np.zeros((8128, NF), np.float32)
    Wd[q, k_] = w

    xt_bf = xt.astype(NPBF16)

    in_maps = []
    for c in range(8):
        big = np.zeros((P, NCOLS), np.float32)
        xsm = np.zeros((P, 2 * NCLASS), np.float32)
        for t in range(NCLASS):
            i = 8 * t + c
            if i > 126:
                continue
            xsm[:, t] = xf[:P, i]
            xsm[:, NCLASS + t] = xf[P:, i]
            p0 = int(rsp[i])
            # cols j in [i,127) hold Wd rows p0..p0+(127-i); leading j in
            # [8t, i) and trailing j=127 stay zero
            o = int(OFFS[t])
            big[:NF, o + (i - 8 * t): o + (127 - 8 * t)] = Wd[p0:p0 + (NF - i)].T
        m = {"xt": xt_bf, "xb": xbm, "xs": xsm, "wd": big.astype(NPBF16)}
        in_maps.append(m)
    return in_maps


def _get_nc():
    if "nc" not in _CACHE:
        _CACHE["nc"] = _build_nc()
    return _CACHE["nc"]


def run_spmd(x, weights, comb_idx, trace=False):
    nc = _get_nc()
    in_maps = _prep_inputs(x, weights, comb_idx)
    res = run_bass_kernel_spmd(nc, in_maps, list(range(8)), trace=trace)
    acc = np.zeros((B, 1), np.float64)
    for c in range(8):
        r = res.results[c]["out"].astype(np.float64)   # [128, 2]
        acc[:P, 0] += r[:, 0]
        acc[P:, 0] += r[:, 1]
    return acc.astype(np.float32), res


def kernel(x, weights, comb_idx):
    out, _ = run_spmd(x, weights, comb_idx, trace=False)
    return out


# revision 9
# speedup vs baseline: 1.3671x; 1.0184x over previous
"""HONU order-3 kernel for 8 TRN2 NeuronCores — raw bass (no TileContext).

Math: out[b] = sum_{i<=j<=k} w_ijk * xf_i * xf_j * xf_k,  xf = [1, x] (127 feats).

Restructuring: group combos by pair (i,j) (lex order => per-pair weights are a
contiguous slice of `weights`).  Let W[(i,j), k] = w_ijk for k>=j (0 otherwise).
Then  Z[b,(i,j)] = sum_k W[(i,j),k] * xf[b,k]   (a dense matmul), and
      out[b]     = sum_i xf_i * sum_{j>=i} xf_j * Z[b,(i,j)]
mapped onto one fused scalar_tensor_tensor per i-row and batch tile:
      g[:, t] = sum_j ((Z * xf_i) * xf_j).

Sharding: pair-rows i are dealt round-robin to the 8 cores (core c gets rows
i = 8t + c, t = 0..15); every core runs the same (SPMD) program and returns a
[128, 2] partial (batch-tile column-major) that the host sums across cores.

Why raw bass: the TileContext version spent ~10us in framework preamble +
semaphore-teardown epilogue (50+ sems cleared one-by-one) and ~4us issuing 7
small DMAs.  Here: 5 consolidated DMAs (>=512B per-partition rows), 7 manual
semaphores cleared with one range-clear, bf16 matmul operands (halves weight
DMA; full-rate PE), fp32 elementwise (STT has no 2x bf16 mode - measured).
"""

import numpy as np
import ml_dtypes

import concourse.bass as bass
import concourse.bacc as bacc
import concourse.mybir as mybir
from concourse.bass_utils import run_bass_kernel_spmd

F32 = mybir.dt.float32
BF16 = mybir.dt.bfloat16
NPBF16 = np.dtype(ml_dtypes.bfloat16)

P = 128
NF = 127            # features incl. bias
B = 256             # batch
NCLASS = 16         # width classes (i-rows per core)
WIDTHS = [128 - 8 * t for t in range(NCLASS)]           # 128,120,...,8
OFFS = np.concatenate([[0], np.cumsum(WIDTHS)])          # class col offsets
NCOLS = int(OFFS[-1])                                    # 1088
# chunk = (class range); each chunk is one matmul (N<=512, one PSUM bank)
CHUNKS = [(0, 4), (4, 9), (9, 16)]
CHUNK_COLS = [int(OFFS[hi] - OFFS[lo]) for lo, hi in CHUNKS]      # 464,400,224
# class -> chunk index
CLASS_CHUNK = {t: ci for ci, (lo, hi) in enumerate(CHUNKS) for t in range(lo, hi)}

_CACHE = {}


def _build_nc():
    nc = bacc.Bacc("TRN2", target_bir_lowering=False, debug=False)
    xt = nc.dram_tensor("xt", [P, B], BF16, kind="ExternalInput")      # xf^T padded
    wd = nc.dram_tensor("wd", [P, NCOLS], BF16, kind="ExternalInput")  # dense pair weights
    xb = nc.dram_tensor("xb", [P, B], F32, kind="ExternalInput")       # xf rows, 2 tiles
    xs = nc.dram_tensor("xs", [P, 2 * NCLASS], F32, kind="ExternalInput")
    out = nc.dram_tensor("out", [P, 2], F32, kind="ExternalOutput")

    from contextlib import ExitStack
    with ExitStack() as ctx:
        xt_t = ctx.enter_context(nc.sbuf_tensor("xt_t", [P, B], BF16))
        wd_t = ctx.enter_context(nc.sbuf_tensor("wd_t", [P, NCOLS], BF16))
        xb_t = ctx.enter_context(nc.sbuf_tensor("xb_t", [P, B], F32))
        xs_t = ctx.enter_context(nc.sbuf_tensor("xs_t", [P, 2 * NCLASS], F32))
        z0_sb = ctx.enter_context(nc.sbuf_tensor("z0_sb", [P, NCOLS], F32))
        z1_sb = ctx.enter_context(nc.sbuf_tensor("z1_sb", [P, NCOLS], F32))
        s_t = ctx.enter_context(nc.sbuf_tensor("s_t", [P, P], F32))
        g_t = ctx.enter_context(nc.sbuf_tensor("g_t", [P, 2 * NCLASS], F32))
        res_t = ctx.enter_context(nc.sbuf_tensor("res_t", [P, 2], F32))
        z00 = ctx.enter_context(nc.psum_tensor("z00", [P, CHUNK_COLS[0]], F32))
        z01 = ctx.enter_context(nc.psum_tensor("z01", [P, CHUNK_COLS[1]], F32))
        z02 = ctx.enter_context(nc.psum_tensor("z02", [P, CHUNK_COLS[2]], F32))
        z10 = ctx.enter_context(nc.psum_tensor("z10", [P, CHUNK_COLS[0]], F32))
        z11 = ctx.enter_context(nc.psum_tensor("z11", [P, CHUNK_COLS[1]], F32))
        z12 = ctx.enter_context(nc.psum_tensor("z12", [P, CHUNK_COLS[2]], F32))
        s_xt = ctx.enter_context(nc.semaphore("s_xt"))
        s_wd = ctx.enter_context(nc.semaphore("s_wd"))
        s_xbs = ctx.enter_context(nc.semaphore("s_xbs"))
        s_mm = ctx.enter_context(nc.semaphore("s_mm"))
        s_act = ctx.enter_context(nc.semaphore("s_act"))
        s_dve = ctx.enter_context(nc.semaphore("s_dve"))
        s_out = ctx.enter_context(nc.semaphore("s_out"))

        z_sb = [z0_sb, z1_sb]
        z_ps = [[z00, z01, z02], [z10, z11, z12]]

        # No nc.Block(): engines end independently (no final all-engine
        # barrier), so the walrus per-engine semaphore-file reset epilogue
        # (~50 sem writes per engine) overlaps the DVE phase on the engines
        # that finish early instead of trailing the whole kernel.  It also
        # re-zeroes our 7 sems for the next execution.
        c0 = CHUNK_COLS[0]

        # DMA issues (sync ring: weights; scalar ring: xb + xs + xt)
        nc.sync.dma_start(wd_t[:, 0:c0], wd[:, 0:c0]).then_inc(s_wd, 16)
        nc.scalar.dma_start(xb_t[:], xb[:]).then_inc(s_xbs, 16)
        nc.sync.dma_start(wd_t[:, c0:NCOLS], wd[:, c0:NCOLS]).then_inc(s_wd, 16)
        nc.scalar.dma_start(xs_t[:], xs[:]).then_inc(s_xbs, 16)
        nc.scalar.dma_start(xt_t[:], xt[:]).then_inc(s_xt, 16)

        # PE: 6 matmuls
        nc.tensor.wait_ge(s_xt, 16)
        for bt in range(2):
            for ci in range(3):
                n = CHUNK_COLS[ci]
                o = int(OFFS[CHUNKS[ci][0]])
                if bt == 0 and ci == 0:
                    nc.tensor.wait_ge(s_wd, 16)
                elif bt == 0 and ci == 1:
                    nc.tensor.wait_ge(s_wd, 32)
                nc.tensor.matmul(
                    z_ps[bt][ci][:],
                    xt_t[:, bt * P:(bt + 1) * P],
                    wd_t[:, o:o + n],
                    start=True, stop=True,
                ).then_inc(s_mm, 1)

        # ACT: PSUM->SBUF chunk copies only for chunks whose classes read
        # SBUF (tile-0 chunks 0-1 are consumed straight from PSUM by the DVE
        # to cut the pipeline-fill latency; their copies are skipped).
        PSUM_DIRECT = {(0, 0), (0, 1)}
        act_idx = {}
        for bt in range(2):
            for ci in range(3):
                if (bt, ci) in PSUM_DIRECT:
                    continue
                n = CHUNK_COLS[ci]
                o = int(OFFS[CHUNKS[ci][0]])
                nc.scalar.wait_ge(s_mm, bt * 3 + ci + 1)
                nc.scalar.copy(z_sb[bt][:, o:o + n], z_ps[bt][ci][:]).then_inc(
                    s_act, 1
                )
                act_idx[(bt, ci)] = len(act_idx) + 1

        # DVE: 32 fused per-class ops + one reduce
        nc.vector.wait_ge(s_xbs, 32)
        for bt in range(2):
            for t in range(NCLASS):
                w = WIDTHS[t]
                o = int(OFFS[t])
                ci = CLASS_CHUNK[t]
                o_chunk = int(OFFS[CHUNKS[ci][0]])
                if t in (0, 4, 9):
                    if (bt, ci) in PSUM_DIRECT:
                        nc.vector.wait_ge(s_mm, bt * 3 + ci + 1)
                    else:
                        nc.vector.wait_ge(s_act, act_idx[(bt, ci)])
                if (bt, ci) in PSUM_DIRECT:
                    in0 = z_ps[bt][ci][:, o - o_chunk:o - o_chunk + w]
                else:
                    in0 = z_sb[bt][:, o:o + w]
                nc.vector.scalar_tensor_tensor(
                    out=s_t[:, :w],
                    in0=in0,
                    scalar=xs_t[:, bt * NCLASS + t:bt * NCLASS + t + 1],
                    in1=xb_t[:, bt * P + 8 * t:bt * P + 8 * t + w],
                    op0=mybir.AluOpType.mult,
                    op1=mybir.AluOpType.mult,
                    accum_out=g_t[:, bt * NCLASS + t:bt * NCLASS + t + 1],
                )
        nc.vector.reduce_sum(
            res_t[:],
            g_t[:].rearrange("p (b t) -> p b t", b=2),
            axis=mybir.AxisListType.X,
        ).then_inc(s_dve, 1)

        # output DMA; completion is guaranteed by the NEFF epilogue's
        # per-engine DMA drain, so no explicit s_out wait.
        nc.sync.wait_ge(s_dve, 1)
        nc.sync.dma_start(out[:], res_t[:]).then_inc(s_out, 16)

    nc.compile()
    return nc


def _prep_inputs(x, weights, comb_idx):
    """Host-side layout prep (no FLOPs on the runtime data beyond zero-fill
    scatter): build xf paddings and the per-core dense weight chunks."""
    x = np.ascontiguousarray(np.asarray(x, dtype=np.float32))
    w = np.asarray(weights, dtype=np.float32).ravel()
    ci = np.asarray(comb_idx)
    i_, j_ = ci[:, 0].astype(np.int64), ci[:, 1].astype(np.int64)
    k_ = ci[:, 2].astype(np.int64)

    xf = np.concatenate([np.ones((B, 1), np.float32), x], axis=1)   # [256,127]
    xbm = np.zeros((P, B), np.float32)       # row p: xf[p,:] | xf[128+p,:]
    xbm[:, :NF] = xf[:P, :]
    xbm[:, P:P + NF] = xf[P:, :]
    xt = np.zeros((P, B), np.float32)
    xt[:NF, :] = xf.T

    # lex pair-row index of each combo
    ar = np.arange(NF, dtype=np.int64)
    rsp = ar * NF - (ar * (ar - 1)) // 2
    q = rsp[i_] + (j_ - i_)
    Wd = np.zeros((8128, NF), np.float32)
    Wd[q, k_] = w

    xt_bf = xt.astype(NPBF16)

    in_maps = []
    for c in range(8):
        big = np.zeros((P, NCOLS), np.float32)
        xsm = np.zeros((P, 2 * NCLASS), np.float32)
        for t in range(NCLASS):
            i = 8 * t + c
            if i > 126:
                continue
            xsm[:, t] = xf[:P, i]
            xsm[:, NCLASS + t] = xf[P:, i]
            p0 = int(rsp[i])
            # cols j in [i,127) hold Wd rows p0..p0+(127-i); leading j in
            # [8t, i) and trailing j=127 stay zero
            o = int(OFFS[t])
            big[:NF, o + (i - 8 * t): o + (127 - 8 * t)] = Wd[p0:p0 + (NF - i)].T
        m = {"xt": xt_bf, "xb": xbm, "xs": xsm, "wd": big.astype(NPBF16)}
        in_maps.append(m)
    return in_maps


def _get_nc():
    if "nc" not in _CACHE:
        _CACHE["nc"] = _build_nc()
    return _CACHE["nc"]


def run_spmd(x, weights, comb_idx, trace=False):
    nc = _get_nc()
    in_maps = _prep_inputs(x, weights, comb_idx)
    res = run_bass_kernel_spmd(nc, in_maps, list(range(8)), trace=trace)
    acc = np.zeros((B, 1), np.float64)
    for c in range(8):
        r = res.results[c]["out"].astype(np.float64)   # [128, 2]
        acc[:P, 0] += r[:, 0]
        acc[P:, 0] += r[:, 1]
    return acc.astype(np.float32), res


def kernel(x, weights, comb_idx):
    out, _ = run_spmd(x, weights, comb_idx, trace=False)
    return out


# revision 10
# speedup vs baseline: 1.4996x; 1.0969x over previous
"""HONU order-3 kernel for 8 TRN2 NeuronCores — raw bass (no TileContext).

Math: out[b] = sum_{i<=j<=k} w_ijk * xf_i * xf_j * xf_k,  xf = [1, x] (127 feats).

Restructuring: group combos by pair (i,j) (lex order => per-pair weights are a
contiguous slice of `weights`).  Let W[(i,j), k] = w_ijk for k>=j (0 otherwise).
Then  Z[b,(i,j)] = sum_k W[(i,j),k] * xf[b,k]   (a dense matmul), and
      out[b]     = sum_i xf_i * sum_{j>=i} xf_j * Z[b,(i,j)]
mapped onto one fused scalar_tensor_tensor per i-row and batch tile:
      g[:, t] = sum_j ((Z * xf_i) * xf_j).

Sharding: pair-rows i are dealt round-robin to the 8 cores (core c gets rows
i = 8t + c, t = 0..15); every core runs the same (SPMD) program and returns a
[128, 2] partial (batch-tile column-major) that the host sums across cores.

Why raw bass: the TileContext version spent ~10us in framework preamble +
semaphore-teardown epilogue (50+ sems cleared one-by-one) and ~4us issuing 7
small DMAs.  Here: 5 consolidated DMAs (>=512B per-partition rows), 7 manual
semaphores cleared with one range-clear, bf16 matmul operands (halves weight
DMA; full-rate PE), fp32 elementwise (STT has no 2x bf16 mode - measured).
"""

import numpy as np
import ml_dtypes

import concourse.bass as bass
import concourse.bacc as bacc
import concourse.mybir as mybir
from concourse.bass_utils import run_bass_kernel_spmd

F32 = mybir.dt.float32
BF16 = mybir.dt.bfloat16
NPBF16 = np.dtype(ml_dtypes.bfloat16)

P = 128
NF = 127            # features incl. bias
B = 256             # batch
NCLASS = 16         # width classes (i-rows per core)
WIDTHS = [128 - 8 * t for t in range(NCLASS)]           # 128,120,...,8
OFFS = np.concatenate([[0], np.cumsum(WIDTHS)])          # class col offsets
NCOLS = int(OFFS[-1])                                    # 1088
# chunk = (class range); each chunk is one matmul (N<=512, one PSUM bank)
CHUNKS = [(0, 4), (4, 9), (9, 16)]
CHUNK_COLS = [int(OFFS[hi] - OFFS[lo]) for lo, hi in CHUNKS]      # 464,400,224
# class -> chunk index
CLASS_CHUNK = {t: ci for ci, (lo, hi) in enumerate(CHUNKS) for t in range(lo, hi)}

_CACHE = {}


def _build_nc():
    nc = bacc.Bacc("TRN2", target_bir_lowering=False, debug=False)
    xt = nc.dram_tensor("xt", [P, B], BF16, kind="ExternalInput")      # xf^T padded
    wd = nc.dram_tensor("wd", [P, NCOLS], BF16, kind="ExternalInput")  # dense pair weights
    xb = nc.dram_tensor("xb", [P, B], F32, kind="ExternalInput")       # xf rows, 2 tiles
    xs = nc.dram_tensor("xs", [P, 2 * NCLASS], F32, kind="ExternalInput")
    out = nc.dram_tensor("out", [P, 2], F32, kind="ExternalOutput")

    from contextlib import ExitStack
    with ExitStack() as ctx:
        xt_t = ctx.enter_context(nc.sbuf_tensor("xt_t", [P, B], BF16))
        wd_t = ctx.enter_context(nc.sbuf_tensor("wd_t", [P, NCOLS], BF16))
        xb_t = ctx.enter_context(nc.sbuf_tensor("xb_t", [P, B], F32))
        xs_t = ctx.enter_context(nc.sbuf_tensor("xs_t", [P, 2 * NCLASS], F32))
        z0_sb = ctx.enter_context(nc.sbuf_tensor("z0_sb", [P, NCOLS], F32))
        z1_sb = ctx.enter_context(nc.sbuf_tensor("z1_sb", [P, NCOLS], F32))
        s_t = ctx.enter_context(nc.sbuf_tensor("s_t", [P, P], F32))
        g_t = ctx.enter_context(nc.sbuf_tensor("g_t", [P, 2 * NCLASS], F32))
        res_t = ctx.enter_context(nc.sbuf_tensor("res_t", [P, 2], F32))
        z00 = ctx.enter_context(nc.psum_tensor("z00", [P, CHUNK_COLS[0]], F32))
        z01 = ctx.enter_context(nc.psum_tensor("z01", [P, CHUNK_COLS[1]], F32))
        z02 = ctx.enter_context(nc.psum_tensor("z02", [P, CHUNK_COLS[2]], F32))
        z10 = ctx.enter_context(nc.psum_tensor("z10", [P, CHUNK_COLS[0]], F32))
        z11 = ctx.enter_context(nc.psum_tensor("z11", [P, CHUNK_COLS[1]], F32))
        z12 = ctx.enter_context(nc.psum_tensor("z12", [P, CHUNK_COLS[2]], F32))
        s_xt = ctx.enter_context(nc.semaphore("s_xt"))
        s_wd = ctx.enter_context(nc.semaphore("s_wd"))
        s_xbs = ctx.enter_context(nc.semaphore("s_xbs"))
        s_mm = ctx.enter_context(nc.semaphore("s_mm"))
        s_act = ctx.enter_context(nc.semaphore("s_act"))
        s_dve = ctx.enter_context(nc.semaphore("s_dve"))
        s_out = ctx.enter_context(nc.semaphore("s_out"))

        z_sb = [z0_sb, z1_sb]
        z_ps = [[z00, z01, z02], [z10, z11, z12]]

        # No nc.Block(): engines end independently (no final all-engine
        # barrier), so the walrus per-engine semaphore-file reset epilogue
        # (~50 sem writes per engine) overlaps the DVE phase on the engines
        # that finish early instead of trailing the whole kernel.  It also
        # re-zeroes our 7 sems for the next execution.
        c0 = CHUNK_COLS[0]

        # DMA issues (sync ring: weights; scalar ring: xt + xs + xb)
        nc.sync.dma_start(wd_t[:, 0:c0], wd[:, 0:c0]).then_inc(s_wd, 16)
        nc.scalar.dma_start(xt_t[:], xt[:]).then_inc(s_xt, 16)
        nc.sync.dma_start(wd_t[:, c0:NCOLS], wd[:, c0:NCOLS]).then_inc(s_wd, 16)
        nc.scalar.dma_start(xs_t[:], xs[:]).then_inc(s_xbs, 16)
        nc.scalar.dma_start(xb_t[:], xb[:]).then_inc(s_xbs, 16)

        # PE: 6 matmuls
        nc.tensor.wait_ge(s_xt, 16)
        for bt in range(2):
            for ci in range(3):
                n = CHUNK_COLS[ci]
                o = int(OFFS[CHUNKS[ci][0]])
                if bt == 0 and ci == 0:
                    nc.tensor.wait_ge(s_wd, 16)
                elif bt == 0 and ci == 1:
                    nc.tensor.wait_ge(s_wd, 32)
                nc.tensor.matmul(
                    z_ps[bt][ci][:],
                    xt_t[:, bt * P:(bt + 1) * P],
                    wd_t[:, o:o + n],
                    start=True, stop=True,
                ).then_inc(s_mm, 1)

        # ACT: PSUM->SBUF chunk copies only for chunks whose classes read
        # SBUF (tile-0 chunks 0-1 are consumed straight from PSUM by the DVE
        # to cut the pipeline-fill latency; their copies are skipped).
        PSUM_DIRECT = {(0, 0), (0, 1)}
        act_idx = {}
        for bt in range(2):
            for ci in range(3):
                if (bt, ci) in PSUM_DIRECT:
                    continue
                n = CHUNK_COLS[ci]
                o = int(OFFS[CHUNKS[ci][0]])
                nc.scalar.wait_ge(s_mm, bt * 3 + ci + 1)
                nc.scalar.copy(z_sb[bt][:, o:o + n], z_ps[bt][ci][:]).then_inc(
                    s_act, 1
                )
                act_idx[(bt, ci)] = len(act_idx) + 1

        # DVE: 32 fused per-class ops + one reduce
        nc.vector.wait_ge(s_xbs, 32)
        for bt in range(2):
            for t in range(NCLASS):
                w = WIDTHS[t]
                o = int(OFFS[t])
                ci = CLASS_CHUNK[t]
                o_chunk = int(OFFS[CHUNKS[ci][0]])
                if t in (0, 4, 9):
                    if (bt, ci) in PSUM_DIRECT:
                        nc.vector.wait_ge(s_mm, bt * 3 + ci + 1)
                    else:
                        nc.vector.wait_ge(s_act, act_idx[(bt, ci)])
                if (bt, ci) in PSUM_DIRECT:
                    in0 = z_ps[bt][ci][:, o - o_chunk:o - o_chunk + w]
                else:
                    in0 = z_sb[bt][:, o:o + w]
                nc.vector.scalar_tensor_tensor(
                    out=s_t[:, :w],
                    in0=in0,
                    scalar=xs_t[:, bt * NCLASS + t:bt * NCLASS + t + 1],
                    in1=xb_t[:, bt * P + 8 * t:bt * P + 8 * t + w],
                    op0=mybir.AluOpType.mult,
                    op1=mybir.AluOpType.mult,
                    accum_out=g_t[:, bt * NCLASS + t:bt * NCLASS + t + 1],
                )
        nc.vector.reduce_sum(
            res_t[:],
            g_t[:].rearrange("p (b t) -> p b t", b=2),
            axis=mybir.AxisListType.X,
        ).then_inc(s_dve, 1)

        # output DMA; completion is guaranteed by the NEFF epilogue's
        # per-engine DMA drain, so no explicit s_out wait.
        nc.sync.wait_ge(s_dve, 1)
        nc.sync.dma_start(out[:], res_t[:]).then_inc(s_out, 16)

    nc.compile()
    return nc


def _prep_inputs(x, weights, comb_idx):
    """Host-side layout prep (no FLOPs on the runtime data beyond zero-fill
    scatter): build xf paddings and the per-core dense weight chunks."""
    x = np.ascontiguousarray(np.asarray(x, dtype=np.float32))
    w = np.asarray(weights, dtype=np.float32).ravel()
    ci = np.asarray(comb_idx)
    i_, j_ = ci[:, 0].astype(np.int64), ci[:, 1].astype(np.int64)
    k_ = ci[:, 2].astype(np.int64)

    xf = np.concatenate([np.ones((B, 1), np.float32), x], axis=1)   # [256,127]
    xbm = np.zeros((P, B), np.float32)       # row p: xf[p,:] | xf[128+p,:]
    xbm[:, :NF] = xf[:P, :]
    xbm[:, P:P + NF] = xf[P:, :]
    xt = np.zeros((P, B), np.float32)
    xt[:NF, :] = xf.T

    # lex pair-row index of each combo
    ar = np.arange(NF, dtype=np.int64)
    rsp = ar * NF - (ar * (ar - 1)) // 2
    q = rsp[i_] + (j_ - i_)
    Wd = np.zeros((8128, NF), np.float32)
    Wd[q, k_] = w

    xt_bf = xt.astype(NPBF16)

    in_maps = []
    for c in range(8):
        big = np.zeros((P, NCOLS), np.float32)
        xsm = np.zeros((P, 2 * NCLASS), np.float32)
        for t in range(NCLASS):
            i = 8 * t + c
            if i > 126:
                continue
            xsm[:, t] = xf[:P, i]
            xsm[:, NCLASS + t] = xf[P:, i]
            p0 = int(rsp[i])
            # cols j in [i,127) hold Wd rows p0..p0+(127-i); leading j in
            # [8t, i) and trailing j=127 stay zero
            o = int(OFFS[t])
            big[:NF, o + (i - 8 * t): o + (127 - 8 * t)] = Wd[p0:p0 + (NF - i)].T
        m = {"xt": xt_bf, "xb": xbm, "xs": xsm, "wd": big.astype(NPBF16)}
        in_maps.append(m)
    return in_maps


def _get_nc():
    if "nc" not in _CACHE:
        _CACHE["nc"] = _build_nc()
    return _CACHE["nc"]


def run_spmd(x, weights, comb_idx, trace=False):
    nc = _get_nc()
    in_maps = _prep_inputs(x, weights, comb_idx)
    res = run_bass_kernel_spmd(nc, in_maps, list(range(8)), trace=trace)
    acc = np.zeros((B, 1), np.float64)
    for c in range(8):
        r = res.results[c]["out"].astype(np.float64)   # [128, 2]
        acc[:P, 0] += r[:, 0]
        acc[P:, 0] += r[:, 1]
    return acc.astype(np.float32), res


def kernel(x, weights, comb_idx):
    out, _ = run_spmd(x, weights, comb_idx, trace=False)
    return out
